# revision 15
# baseline (speedup 1.0000x reference)
"""Distributed multi-head attention kernel for one TRN2 chip (8 NeuronCores).

Problem: b=2, n=2048, dim=1024, heads=16, hd=64.
  qkv = x @ Wqkv.T  (qkv-major split) -> RoPE(q,k) -> softmax(q k^T/8) v
  -> merge heads -> @ Wproj.T + bproj

Sharding: each core owns 2 heads (of 16) for BOTH batches. QKV projection,
RoPE and attention are fully head-local. One 8-way AllToAll redistributes
attention outputs from head-major to token-major; each core then projects a
512-token slice of the flattened (b*n) axis. Host concatenates slices.

Per-core inputs (see make_in_maps):
  x        [4096, 1024] bf16  flat tokens x channels, replicated
  wqkv     [384, 1024]  bf16  q(2x64) | k(128) | v(128) rows for my heads
  wproj    [1024, 1024] bf16  full, replicated
  bproj    [1, 1024]    f32
  sin/cos  [2048, 64]   f32   (position = flat_token % 2048)
  out      [512, 1024]  f32   flat tokens [512c : 512(c+1)]

All matmuls bf16 (PSUM accumulates f32). scoresT layout [k_j, q_i] (k
stationary, both heads row-packed in the 128-partition contraction) so the
softmax needs no transposes: denominators come from a ones-column appended to
v. exp on ScalarE with fused 1/8 scale; no max subtraction (scores std ~2).
Transposes ride the DMA xbar in large batched ops straight from DRAM.
"""

import os
import numpy as np

NUM_CORES = 8
B, N, DIM, NH, HD = 2, 2048, 1024, 16, 64
T = B * N                 # 4096 flat tokens
HPC = NH // NUM_CORES     # 2 heads per core
P = 128
CT = DIM // P             # 8 channel tiles
TSLICE = T // NUM_CORES   # 512 output tokens per core
QW = HPC * HD             # 128
FQKV = 3 * QW             # 384

_CACHE = {}


def _build_nc():
    from concourse import bacc, mybir, tile

    f32 = mybir.dt.float32
    bf16 = mybir.dt.bfloat16
    Exp = mybir.ActivationFunctionType.Exp
    mult = mybir.AluOpType.mult
    add = mybir.AluOpType.add

    nc = bacc.Bacc("TRN2", target_bir_lowering=False, debug=False,
                   num_devices=NUM_CORES)

    x_d = nc.dram_tensor("x", [T, DIM], bf16, kind="ExternalInput")
    wqkv_d = nc.dram_tensor("wqkv", [FQKV, DIM], bf16, kind="ExternalInput")
    wproj_d = nc.dram_tensor("wproj", [DIM, DIM], bf16, kind="ExternalInput")
    bproj_d = nc.dram_tensor("bproj", [1, DIM], f32, kind="ExternalInput")
    sin_d = nc.dram_tensor("sin", [N, HD], f32, kind="ExternalInput")
    cos_d = nc.dram_tensor("cos", [N, HD], f32, kind="ExternalInput")
    out_d = nc.dram_tensor("out", [TSLICE, DIM], f32, kind="ExternalOutput")
    a2a_in = nc.dram_tensor("a2a_in", [NUM_CORES * P, TSLICE], bf16)
    a2a_out = nc.dram_tensor("a2a_out", [NUM_CORES * P, TSLICE], bf16)

    with tile.TileContext(nc) as tc:
        with (
            tc.tile_pool(name="persist", bufs=1) as pers,
            tc.tile_pool(name="work", bufs=3) as wp,
            tc.tile_pool(name="expp", bufs=3) as ep,
            tc.tile_pool(name="psA", bufs=2, space="PSUM") as psA,   # qkv/bc/proj
            tc.tile_pool(name="psS", bufs=2, space="PSUM") as psS,   # scores
            tc.tile_pool(name="psV", bufs=1, space="PSUM") as psV,   # av accum
        ):
            # ---------------- persistent SBUF ----------------
            wqkvT = pers.tile([P, CT * FQKV], bf16)     # ct-block: [128c, 384f]
            wprojT = pers.tile([P, CT * DIM], bf16)     # dt-block: [128d', 1024f]
            xT = pers.tile([P, CT * N], bf16)           # ct-block: [128c, 2048t] (half)
            qT = pers.tile([P, T], bf16)                # [d(2 heads), flat t]
            kT = pers.tile([P, T], bf16)
            v_sb = pers.tile([P, HPC * (T // P) * 65], bf16)
            aoT = pers.tile([P, T], bf16)               # [d', flat t]
            aoTr = pers.tile([P, T], bf16)              # post-A2A [d' chunk, t]
            sin4 = pers.tile([P, 16 * 4 * HD], bf16)
            cos4 = pers.tile([P, 16 * 4 * HD], bf16)
            sneg4 = pers.tile([P, 16 * 4 * HD], bf16)
            ones_col = pers.tile([1, P], bf16)
            bias_bf = pers.tile([1, DIM], bf16)

            nc.vector.memset(ones_col, 1.0)
            nc.vector.memset(v_sb, 1.0)                 # ones cols survive

            # ---------------- weight / bias / sincos prep ----------------
            # wqkvT blocks straight from DRAM via xbar transpose
            for ct in range(CT):
                nc.sync.dma_start_transpose(
                    wqkvT[:, FQKV * ct:FQKV * (ct + 1)],
                    wqkv_d[:, P * ct:P * (ct + 1)])

            bt = wp.tile([1, DIM], f32, tag="bload")
            nc.sync.dma_start(bt, bproj_d[:, :])
            nc.vector.tensor_copy(bias_bf, bt)

            sin_f = wp.tile([P, 16 * HD], f32, tag="scload")
            cos_f = wp.tile([P, 16 * HD], f32, tag="scload2")
            for pt in range(16):
                nc.sync.dma_start(sin_f[:, HD * pt:HD * (pt + 1)],
                                    sin_d[P * pt:P * (pt + 1), :])
                nc.sync.dma_start(cos_f[:, HD * pt:HD * (pt + 1)],
                                    cos_d[P * pt:P * (pt + 1), :])
            s4 = sin4.rearrange("p (pt c d) -> p pt c d", pt=16, c=4)
            c4 = cos4.rearrange("p (pt c d) -> p pt c d", pt=16, c=4)
            n4 = sneg4.rearrange("p (pt c d) -> p pt c d", pt=16, c=4)
            sf = sin_f.rearrange("p (pt d) -> p pt d", pt=16)
            cf = cos_f.rearrange("p (pt d) -> p pt d", pt=16)
            for c in range(4):
                nc.vector.tensor_copy(s4[:, :, c, :], sf)
                nc.vector.tensor_copy(c4[:, :, c, :], cf)
            nc.vector.tensor_scalar_mul(n4[:, :, :, 0:32], s4[:, :, :, 0:32], -1.0)
            nc.vector.tensor_copy(n4[:, :, :, 32:64], s4[:, :, :, 32:64])

            QB = 512
            TTH = N // P            # 16 token tiles per batch half

            for b in range(B):
                # ---------- xT for this batch half (from DRAM, xbar) ----------
                for ct in range(CT):
                    nc.sync.dma_start_transpose(
                        xT[:, N * ct:N * (ct + 1)],
                        x_d[N * b:N * (b + 1), P * ct:P * (ct + 1)])

                # ---------- QKV + RoPE, groups of 4 token tiles ----------
                for g in range(TTH // 4):
                    qbuf = wp.tile([P, 4 * P], bf16, tag="qbuf")
                    kbuf = wp.tile([P, 4 * P], bf16, tag="kbuf")
                    for ttl in range(4):
                        tt = 4 * g + ttl            # tile within batch half
                        ftt = TTH * b + tt          # flat tile
                        qkvp = psA.tile([P, 512], f32, tag="mm", name="qkvp")
                        for ct in range(CT):
                            nc.tensor.matmul(
                                qkvp[:, 0:FQKV],
                                xT[:, N * ct + P * tt:N * ct + P * (tt + 1)],
                                wqkvT[:, FQKV * ct:FQKV * (ct + 1)],
                                start=(ct == 0), stop=(ct == CT - 1))
                        qkc = wp.tile([P, 2 * QW], bf16, tag="qkc")
                        nc.vector.tensor_copy(qkc, qkvp[:, 0:2 * QW])
                        pt = tt % 16
                        qk3 = qkc.rearrange("p (c d) -> p c d", c=4)
                        t1 = wp.tile([P, 2 * QW], bf16, tag="t1")
                        t13 = t1.rearrange("p (c d) -> p c d", c=4)
                        nc.vector.tensor_tensor(t13[:, :, 0:32], qk3[:, :, 32:64],
                                                n4[:, pt, :, 0:32], mult)
                        nc.vector.tensor_tensor(t13[:, :, 32:64], qk3[:, :, 0:32],
                                                n4[:, pt, :, 32:64], mult)
                        qkcos = wp.tile([P, 2 * QW], bf16, tag="qkcos")
                        nc.vector.tensor_tensor(
                            qkcos, qkc, cos4[:, 4 * HD * pt:4 * HD * (pt + 1)], mult)
                        nc.vector.tensor_tensor(qbuf[:, P * ttl:P * (ttl + 1)],
                                                qkcos[:, 0:QW], t1[:, 0:QW], add)
                        nc.vector.tensor_tensor(kbuf[:, P * ttl:P * (ttl + 1)],
                                                qkcos[:, QW:2 * QW], t1[:, QW:2 * QW],
                                                add)
                        vv = v_sb.rearrange("p (h t e) -> p h t e", h=HPC, t=T // P)
                        nc.vector.tensor_copy(
                            vv[:, :, ftt, 0:HD],
                            qkvp[:, 2 * QW:3 * QW].rearrange("p (h d) -> p h d",
                                                             h=HPC))
                    # batched transposes into qT/kT (4 tiles each)
                    t0 = P * (TTH * b + 4 * g)
                    nc.sync.dma_start_transpose(
                        qT[:, t0:t0 + 4 * P].rearrange("p (c t) -> p c t", c=4), qbuf)
                    nc.sync.dma_start_transpose(
                        kT[:, t0:t0 + 4 * P].rearrange("p (c t) -> p c t", c=4), kbuf)

                if b == 0:
                    # wprojT blocks (needed only at projection time) — fill
                    # sync-queue gaps while attention for b0 runs
                    for dt in range(CT):
                        nc.sync.dma_start_transpose(
                            wprojT[:, DIM * dt:DIM * (dt + 1)],
                            wproj_d[:, P * dt:P * (dt + 1)])

                # ---------- attention for this batch ----------
                for qq in range(N // QB):
                    avp = psV.tile([65, HPC * QB], f32, tag="av", name="avp")
                    av = [avp[:, QB * h:QB * (h + 1)] for h in range(HPC)]
                    for jt in range(TTH):
                        ftt = TTH * b + jt
                        sp = psS.tile([P, HPC * QB], f32, tag="scores")
                        for h in range(HPC):
                            nc.tensor.matmul(
                                sp[:, QB * h:QB * (h + 1)],
                                kT[HD * h:HD * (h + 1), P * ftt:P * (ftt + 1)],
                                qT[HD * h:HD * (h + 1),
                                   N * b + QB * qq:N * b + QB * (qq + 1)],
                                start=True, stop=True)
                        et = ep.tile([P, HPC * QB], bf16, tag="expT")
                        nc.scalar.activation(et, sp, Exp, scale=float(HD) ** -0.5)
                        for h in range(HPC):
                            blk = (h * (T // P) + ftt) * 65
                            nc.tensor.matmul(av[h], v_sb[:, blk:blk + 65],
                                             et[:, QB * h:QB * (h + 1)],
                                             start=(jt == 0), stop=(jt == TTH - 1))
                    # one copy releases the PSUM accumulator; norm runs off SBUF
                    avf = wp.tile([65, HPC * QB], f32, tag="avf")
                    nc.vector.tensor_copy(avf, avp)
                    for h in range(HPC):
                        sums = wp.tile([1, QB], bf16, tag="sums")
                        nc.vector.tensor_copy(sums, avf[64:65, QB * h:QB * (h + 1)])
                        bc = psA.tile([64, QB], f32, tag="mm", name="bc")
                        nc.tensor.matmul(bc, ones_col[:, 0:64], sums,
                                         start=True, stop=True)
                        rc = wp.tile([64, QB], f32, tag="recip")
                        nc.vector.reciprocal_approx_fast(rc, bc)
                        nc.vector.tensor_tensor(
                            aoT[HD * h:HD * (h + 1),
                                N * b + QB * qq:N * b + QB * (qq + 1)],
                            avf[0:64, QB * h:QB * (h + 1)], rc, mult)

            # ---------------- AllToAll + output projection ----------------
            a2i = a2a_in.ap().rearrange("(c p) t -> p c t", p=P)
            nc.sync.dma_start(a2i, aoT.rearrange("p (c t) -> p c t", c=NUM_CORES))
            nc.gpsimd.collective_compute(
                "AllToAll", mybir.AluOpType.bypass,
                replica_groups=[list(range(NUM_CORES))],
                ins=[a2a_in.ap().opt()], outs=[a2a_out.ap().opt()])
            a2o = a2a_out.ap().rearrange("(c p) t -> p c t", p=P)
            nc.sync.dma_start(aoTr.rearrange("p (c t) -> p c t", c=NUM_CORES), a2o)

            for ts in range(TSLICE // P):
                for fb in range(DIM // 512):
                    pp = psA.tile([P, 512], f32, tag="mm", name="proj")
                    for dt in range(CT):
                        nc.tensor.matmul(
                            pp,
                            aoTr[:, TSLICE * dt + P * ts:TSLICE * dt + P * (ts + 1)],
                            wprojT[:, DIM * dt + 512 * fb:DIM * dt + 512 * (fb + 1)],
                            start=(dt == 0), stop=False)
                    nc.tensor.matmul(pp, ones_col[:, 0:P],
                                     bias_bf[:, 512 * fb:512 * (fb + 1)],
                                     start=False, stop=True)
                    ob = wp.tile([P, 512], f32, tag="ob")
                    nc.vector.tensor_copy(ob, pp)
                    nc.sync.dma_start(out_d[P * ts:P * (ts + 1),
                                              512 * fb:512 * (fb + 1)], ob)

    nc.compile()
    return nc


def _get_nc():
    if "nc" not in _CACHE:
        _CACHE["nc"] = _build_nc()
    return _CACHE["nc"]


def make_in_maps(x, Wqkv, Wproj, bproj, sin, cos):
    """Shard full (f32) inputs into per-core in_maps (weights/x cast to bf16)."""
    import ml_dtypes
    bf16 = ml_dtypes.bfloat16
    x = np.ascontiguousarray(np.asarray(x, np.float32).reshape(T, DIM).astype(bf16))
    Wqkv = np.asarray(Wqkv, np.float32).astype(bf16)
    Wproj = np.ascontiguousarray(np.asarray(Wproj, np.float32).astype(bf16))
    bproj = np.asarray(bproj, np.float32).reshape(1, DIM)
    sin = np.asarray(sin, np.float32)
    cos = np.asarray(cos, np.float32)
    in_maps = []
    for c in range(NUM_CORES):
        r = P * c
        wq = Wqkv[r:r + P]
        wk = Wqkv[DIM + r:DIM + r + P]
        wv = Wqkv[2 * DIM + r:2 * DIM + r + P]
        in_maps.append({
            "x": x,
            "wqkv": np.concatenate([wq, wk, wv], 0).copy(),
            "wproj": Wproj,
            "bproj": bproj,
            "sin": sin,
            "cos": cos,
        })
    return in_maps


def kernel(x, Wqkv, Wproj, bproj, sin, cos):
    from concourse.bass_utils import run_bass_kernel_spmd

    nc = _get_nc()
    in_maps = make_in_maps(x, Wqkv, Wproj, bproj, sin, cos)
    trace = bool(int(os.environ.get("KERNEL_TRACE", "0")))
    res = run_bass_kernel_spmd(nc, in_maps, core_ids=list(range(NUM_CORES)),
                               trace=trace)
    _CACHE["last_result"] = res
    out = np.concatenate([res.results[c]["out"] for c in range(NUM_CORES)], 0)
    return out.reshape(B, N, DIM).astype(np.float32)
